# revision 21
# baseline (speedup 1.0000x reference)
"""TP-8 Trainium2 Bass kernel for the Llama2-style greedy-decode problem.

Single NEFF per core, SPMD over 8 cores. Megatron TP-8: qkv/gate/up
column-sharded (2 heads, FF 352 per core), wo/w_down row-sharded
(AllReduce partials), lm_head vocab-sharded (4000 cols/core).
Prefill(128) + 7 KV-cache decode steps, on-device argmax and
indirect-DMA embedding gather. Weights SBUF-resident bf16 (host-cast,
host-prepared layouts, one contiguous DMA each); activations f32.
Decode keeps h in column form [128, 2k+b]; softmax skips max-subtraction
(scores bounded); batched 3D-AP ops throughout.
clr output = logits - mean(logits) (log_softmax centering cancels).
"""
import sys

sys.path.insert(0, "/opt/trn_rl_repo")
import contextlib  # noqa: E402
import numpy as np  # noqa: E402

import concourse.bass as bass  # noqa: E402
import concourse.mybir as mybir  # noqa: E402
import concourse.tile as tile  # noqa: E402
from concourse import bacc, bass_utils  # noqa: E402

F32 = mybir.dt.float32
BF16 = mybir.dt.bfloat16
U32 = mybir.dt.uint32
AX = mybir.AxisListType
AF = mybir.ActivationFunctionType
ALU = mybir.AluOpType

NH, D, FF, NL, B, L, T_NEW, V, HOUT = 16, 1024, 2816, 2, 2, 128, 8, 32000, 1124
EPS = 1e-5
ROPE_BASE = 10000.0
TP = 8
HC = NH // TP          # 2 heads per core
HD = D // NH           # 64
QC = HC * HD           # 128 local qkv cols
FS = FF // TP          # 352
VS = V // TP           # 4000
SMAX = L + T_NEW       # 136
PT = B * L             # 256
KT = D // 128          # 8
RG = [list(range(TP))]
VCW = 500              # vocab chunk width (8 chunks of 500)


def build():
    nc = bacc.Bacc("TRN2", target_bir_lowering=False, debug=False, num_devices=TP)

    def inp(name, shape, dtype=F32):
        return nc.dram_tensor(name, shape, dtype, kind="ExternalInput")

    h0T_in = inp("h0T", [128, KT * PT])               # prefill h, column layout
    wqkv_in = [inp(f"wqkv{l}", [128, KT * 3 * QC], BF16) for l in range(NL)]
    wo_in = [inp(f"wo{l}", [QC, D], BF16) for l in range(NL)]
    wg_in = [inp(f"wg{l}", [128, KT * FS], BF16) for l in range(NL)]
    wu_in = [inp(f"wu{l}", [128, KT * FS], BF16) for l in range(NL)]
    wd_in = [inp(f"wd{l}", [128, 3 * D], BF16) for l in range(NL)]
    lmh_in = inp("lmh", [128, KT * VS], BF16)
    emb_in = inp("emb", [V, D])
    pcosT_in = inp("pcosT", [128, PT])
    psinT_in = inp("psinT", [128, PT])
    dcosW_in = inp("dcosW", [B, (T_NEW - 1) * 4 * HD])
    dsinW_in = inp("dsinW", [B, (T_NEW - 1) * 4 * HD])
    cmask4_in = inp("cmask4", [L, 4 * L])
    idf_in = inp("idf", [128, 128])
    idb_in = inp("idb", [128, 128], BF16)
    coreoff_in = inp("coreoff", [B, 1])
    out_t = nc.dram_tensor("out", [B, T_NEW, HOUT], F32, kind="ExternalOutput")
    dbg_st = nc.dram_tensor("dbg_st", [B, 4], F32, kind="ExternalOutput")
    dbg_gsr = nc.dram_tensor("dbg_gsr", [1, 64], F32, kind="ExternalOutput")
    dbg_nid = nc.dram_tensor("dbg_nid", [B, 1], F32, kind="ExternalOutput")
    dbg_m8 = nc.dram_tensor("dbg_m8", [B, 8], F32, kind="ExternalOutput")
    dbg_ll = nc.dram_tensor("dbg_ll", [B, 1000], F32, kind="ExternalOutput")
    dbg_hT2 = nc.dram_tensor("dbg_hT2", [128, 16], F32, kind="ExternalOutput")

    with tile.TileContext(nc) as tc:
        ctx = contextlib.ExitStack()
        with ctx:
            wp = ctx.enter_context(tc.tile_pool(name="wts", bufs=1))
            cp = ctx.enter_context(tc.tile_pool(name="const", bufs=1))
            kvp = ctx.enter_context(tc.tile_pool(name="kv", bufs=1))
            sb = ctx.enter_context(tc.tile_pool(name="work", bufs=2))
            hb = ctx.enter_context(tc.tile_pool(name="hrows", bufs=2))
            dp = ctx.enter_context(tc.tile_pool(name="dram", bufs=2, space="DRAM"))

            import os as _os0
            KBAR = int(_os0.environ.get("KBAR", "0"))

            @contextlib.contextmanager
            def psum_pool(name, bufs=1):
                with tc.tile_pool(name=name, bufs=bufs, space="PSUM") as p:
                    yield p
                if KBAR:
                    tc.strict_bb_all_engine_barrier()

            # ---- constants ----
            idf = cp.tile([128, 128], F32, tag="idf")
            nc.sync.dma_start(idf[:], idf_in[:])
            idb = cp.tile([128, 128], BF16, tag="idb")
            nc.sync.dma_start(idb[:], idb_in[:])
            ones = cp.tile([128, 1], F32, tag="ones")
            nc.vector.memset(ones[:], 1.0)
            onesr = cp.tile([1, 128], F32, tag="onesr")
            nc.vector.memset(onesr[:], 1.0)
            big18 = cp.tile([1, 8], F32, tag="big18")
            nc.vector.memset(big18[:], 1e12)
            epsc = cp.tile([1, 1], F32, tag="epsc")
            nc.vector.memset(epsc[:], EPS)
            pcosT = cp.tile([128, PT], F32, tag="pcos")
            nc.sync.dma_start(pcosT[:], pcosT_in[:])
            psinT = cp.tile([128, PT], F32, tag="psin")
            nc.sync.dma_start(psinT[:], psinT_in[:])
            dcosW = cp.tile([B, (T_NEW - 1) * 4 * HD], F32, tag="dcos")
            nc.sync.dma_start(dcosW[:], dcosW_in[:])
            dsinW = cp.tile([B, (T_NEW - 1) * 4 * HD], F32, tag="dsin")
            nc.sync.dma_start(dsinW[:], dsinW_in[:])
            cmask4 = cp.tile([L, 4 * L], F32, tag="cmask4")
            nc.sync.dma_start(cmask4[:], cmask4_in[:])
            coreoff = cp.tile([B, 1], F32, tag="coff")
            nc.sync.dma_start(coreoff[:], coreoff_in[:])

            # ---- weights (host-prepared layouts, one DMA each) ----
            wqkv_sb, wo_sb, wg_sb, wu_sb, wd_sb = [], [], [], [], []
            for l in range(NL):
                t = wp.tile([128, KT * 3 * QC], BF16, tag=f"wqkv{l}")
                nc.sync.dma_start(t[:], wqkv_in[l][:])
                wqkv_sb.append(t)
                t = wp.tile([128, D], BF16, tag=f"wo{l}")
                nc.sync.dma_start(t[:], wo_in[l][:])
                wo_sb.append(t)
                t = wp.tile([128, KT * FS], BF16, tag=f"wg{l}")
                nc.sync.dma_start(t[:], wg_in[l][:])
                wg_sb.append(t)
                t = wp.tile([128, KT * FS], BF16, tag=f"wu{l}")
                nc.sync.dma_start(t[:], wu_in[l][:])
                wu_sb.append(t)
                t = wp.tile([128, 3 * D], BF16, tag=f"wd{l}")
                nc.sync.dma_start(t[:], wd_in[l][:])
                wd_sb.append(t)
            lmh_sb = wp.tile([128, KT * VS], BF16, tag="lmh")
            nc.sync.dma_start(lmh_sb[:], lmh_in[:])

            # ---- KV caches ----
            kT_c = [kvp.tile([128, B * SMAX], BF16, tag=f"kT{l}", name=f"kT{l}")
                    for l in range(NL)]
            v0_c = [kvp.tile([128, 4 * HD], BF16, tag=f"v0{l}", name=f"v0{l}")
                    for l in range(NL)]
            v1_c = [kvp.tile([8, 4 * HD], BF16, tag=f"v1{l}", name=f"v1{l}")
                    for l in range(NL)]

            # ================= prefill =================
            # hT column layout: col = k*256 + b*128 + t
            hT = hb.tile([128, KT * PT], F32, tag="hT", name="hT", bufs=1)
            nc.sync.dma_start(hT[:], h0T_in[:])

            def rms_norm_T(h):
                """[128, KT*PT] -> normalized bf16 same layout."""
                sq = sb.tile([128, KT * PT], F32, tag="big8k", name="sq", bufs=1)
                nc.vector.tensor_tensor(out=sq[:], in0=h[:], in1=h[:], op=ALU.mult)
                x = sb.tile([128, KT * PT], BF16, tag="xnorm", name="xnorm", bufs=1)
                with psum_pool("prms") as pp:
                    ssp = pp.tile([1, KT * PT], F32, tag="ssp", name="ssp")
                    for j in range(4):
                        nc.tensor.matmul(out=ssp[0:1, j * 512:(j + 1) * 512],
                                         lhsT=ones[:],
                                         rhs=sq[:, j * 512:(j + 1) * 512],
                                         start=True, stop=True)
                    sums = sb.tile([1, PT], F32, tag="sums", name="sums", bufs=1)
                    nc.vector.tensor_reduce(
                        sums[:], ssp[0:1, :].rearrange("p (k c) -> p c k", k=KT),
                        AX.X, ALU.add)
                    sd = sb.tile([1, PT], F32, tag="sd", name="sd", bufs=1)
                    nc.scalar.activation(sd[:], sums[:], AF.Sqrt, bias=epsc[:],
                                         scale=1.0 / D)
                    r = sb.tile([1, PT], F32, tag="rn", name="rn", bufs=1)
                    nc.vector.reciprocal(r[:], sd[:])
                    rbc = pp.tile([128, PT], F32, tag="rbc", name="rbc")
                    nc.tensor.matmul(out=rbc[:], lhsT=onesr[:], rhs=r[:],
                                     start=True, stop=True)
                    for k in range(KT):
                        nc.vector.tensor_tensor(out=x[:, k * PT:(k + 1) * PT],
                                                in0=h[:, k * PT:(k + 1) * PT],
                                                in1=rbc[:], op=ALU.mult)
                return x

            def ropeT(psrc, cache_dst=None):
                rot = sb.tile([128, PT], F32, tag="rot", name="rot", bufs=1)
                for h in range(HC):
                    b0 = h * HD
                    nc.scalar.activation(rot[b0:b0 + 32, :], psrc[b0 + 32:b0 + 64, :],
                                         AF.Copy)
                    nc.scalar.activation(rot[b0 + 32:b0 + 64, :], psrc[b0:b0 + 32, :],
                                         AF.Copy)
                t1 = sb.tile([128, PT], F32, tag="rt1", name="rt1", bufs=1)
                nc.vector.tensor_tensor(out=t1[:], in0=psrc[:], in1=pcosT[:],
                                        op=ALU.mult)
                nc.vector.tensor_tensor(out=rot[:], in0=rot[:], in1=psinT[:],
                                        op=ALU.mult)
                if cache_dst is None:
                    o = sb.tile([128, PT], BF16, tag="qro", name="qro", bufs=1)
                    nc.vector.tensor_tensor(out=o[:], in0=t1[:], in1=rot[:],
                                            op=ALU.add)
                    return o
                for b in range(B):
                    nc.vector.tensor_tensor(
                        out=cache_dst[:, b * SMAX:b * SMAX + L],
                        in0=t1[:, b * L:(b + 1) * L],
                        in1=rot[:, b * L:(b + 1) * L], op=ALU.add)
                return None

            def ar_big(psum_flat, h):
                """AllReduce [128, KT*PT] partial (512-strided psum) into h."""
                ev = sb.tile([128, KT * PT], F32, tag="big8k", name="aev", bufs=1)
                nc.vector.tensor_copy(
                    ev[:].rearrange("p (k c) -> p k c", k=KT),
                    psum_flat[:].rearrange("p (k c) -> p k c", k=KT)[:, :, 0:PT])
                bi = dp.tile([128, KT * PT], F32, tag="abi", name="abi")
                bo = dp.tile([128, KT * PT], F32, tag="abo", name="abo")
                nc.sync.dma_start(bi[:], ev[:])
                nc.gpsimd.collective_compute("AllReduce", ALU.add, replica_groups=RG,
                                             ins=[bi[:].opt()], outs=[bo[:].opt()])
                g = sb.tile([128, KT * PT], F32, tag="big8k", name="agt", bufs=1)
                nc.sync.dma_start(g[:], bo[:])
                nc.vector.tensor_tensor(out=h[:], in0=h[:], in1=g[:], op=ALU.add)

            import os as _os
            KPREF = int(_os.environ.get("KPREF", "99"))
            for l in range(NL if KPREF > 10 else 1):
                if KPREF == 0:
                    break
                xb = rms_norm_T(hT)
                if KPREF == 1:
                    break
                qb = None
                vb = sb.tile([128, PT], BF16, tag="vb", name="vb", bufs=1)
                with psum_pool("pqkv") as pp:
                    pq3 = []
                    for wi in range(3):
                        ps = pp.tile([128, PT], F32, tag=f"pqkv{wi}",
                                     name=f"pqkv{wi}")
                        for k in range(KT):
                            nc.tensor.matmul(
                                out=ps[:],
                                lhsT=wqkv_sb[l][:, (k * 3 + wi) * QC:
                                                (k * 3 + wi + 1) * QC],
                                rhs=xb[:, k * PT:(k + 1) * PT],
                                start=(k == 0), stop=(k == KT - 1))
                        pq3.append(ps)
                    qb = ropeT(pq3[0])
                    ropeT(pq3[1], cache_dst=kT_c[l])
                    nc.scalar.activation(vb[:], pq3[2][:], AF.Copy)
                if KPREF == 2:
                    break
                # v cache transposed: [L, (2b+h)*HD]
                with psum_pool("pvt", bufs=2) as pp:
                    for b in range(B):
                        for h in range(HC):
                            p = 2 * b + h
                            pv = pp.tile([128, HD], BF16, tag="pvT", name="pvT")
                            nc.tensor.transpose(
                                pv[:],
                                vb[h * HD:(h + 1) * HD, b * L:(b + 1) * L],
                                idb[h * HD:(h + 1) * HD, h * HD:(h + 1) * HD])
                            nc.scalar.activation(v0_c[l][:, p * HD:(p + 1) * HD],
                                                 pv[:], AF.Copy)
                if KPREF == 3:
                    break
                # attention: scores [128, 4*128] (p = 2b+h blocks)
                oT = sb.tile([128, PT], BF16, tag="oT", name="oT", bufs=1)
                esb = sb.tile([L, 4 * L], F32, tag="esb", name="esb", bufs=1)
                with psum_pool("pattn") as pp:
                    psc = pp.tile([L, 4 * 512], F32, tag="psc", name="psc")
                    pscv = psc[:].rearrange("p (g c) -> p g c", g=4)[:, :, 0:L]
                    for b in range(B):
                        for h in range(HC):
                            p = 2 * b + h
                            nc.tensor.matmul(
                                out=psc[:, p * 512:p * 512 + L],
                                lhsT=qb[h * HD:(h + 1) * HD, b * L:(b + 1) * L],
                                rhs=kT_c[l][h * HD:(h + 1) * HD,
                                            b * SMAX:b * SMAX + L],
                                start=True, stop=True)
                    if KPREF == 31:
                        nc.scalar.activation(oT[:, 0:PT], psc[:, 0:PT], AF.Copy)
                        break
                    nc.vector.tensor_tensor(
                        out=pscv, in0=pscv,
                        in1=cmask4[:].rearrange("p (g c) -> p g c", g=4),
                        op=ALU.add)
                    nc.scalar.activation(
                        esb[:].rearrange("p (g c) -> p g c", g=4), pscv,
                        AF.Exp, scale=0.125)
                if KPREF == 32:
                    break
                ssum = sb.tile([L, 4], F32, tag="ssum", name="ssum")
                nc.vector.tensor_reduce(
                    ssum[:], esb[:].rearrange("p (q c) -> p q c", q=4),
                    AX.X, ALU.add)
                rr = sb.tile([L, 4], F32, tag="rrp", name="rrp")
                nc.vector.reciprocal(rr[:], ssum[:])
                att = sb.tile([L, 4 * L], BF16, tag="att", name="att", bufs=1)
                for p in range(4):
                    nc.vector.tensor_scalar_mul(att[:, p * L:(p + 1) * L],
                                                esb[:, p * L:(p + 1) * L],
                                                rr[:, p:p + 1])
                if KPREF == 33:
                    break
                attT = sb.tile([L, 4 * L], BF16, tag="attT", name="attT", bufs=1)
                with psum_pool("patrp", bufs=2) as pp:
                    for p in range(4):
                        paT = pp.tile([L, L], BF16, tag="paT", name="paT")
                        nc.tensor.transpose(paT[:],
                                            att[:, p * L:(p + 1) * L], idb[:L, :L])
                        nc.scalar.activation(attT[:, p * L:(p + 1) * L], paT[:],
                                             AF.Copy)
                if KPREF == 34:
                    break
                with psum_pool("povp") as pp:
                    pov = pp.tile([HD, 4 * 512], F32, tag="pov", name="pov")
                    for p in range(4):
                        nc.tensor.matmul(out=pov[:, p * 512:p * 512 + L],
                                         lhsT=v0_c[l][:, p * HD:(p + 1) * HD],
                                         rhs=attT[:, p * L:(p + 1) * L],
                                         start=True, stop=True)
                    for b in range(B):
                        for h in range(HC):
                            p = 2 * b + h
                            nc.scalar.activation(
                                oT[h * HD:(h + 1) * HD, b * L:(b + 1) * L],
                                pov[:, p * 512:p * 512 + L], AF.Copy)
                with psum_pool("pwop") as pp:
                    pwo = pp.tile([128, KT * 512], F32, tag="pbig", name="pwo")
                    for m in range(KT):
                        nc.tensor.matmul(out=pwo[:, m * 512:m * 512 + PT],
                                         lhsT=wo_sb[l][:, m * 128:(m + 1) * 128],
                                         rhs=oT[:], start=True, stop=True)
                    ar_big(pwo, hT)
                if KPREF == 5:
                    break
                # --- FFN ---
                xb2 = rms_norm_T(hT)
                with psum_pool("pffn") as pp:
                    pg = pp.tile([128, 3 * 512], F32, tag="pgu0", name="pg")
                    pu = pp.tile([128, 3 * 512], F32, tag="pgu1", name="pu")
                    pgv = pg[:].rearrange("p (g c) -> p g c", g=3)[:, :, 0:PT]
                    puv = pu[:].rearrange("p (g c) -> p g c", g=3)[:, :, 0:PT]
                    for ps, wsb in ((pg, wg_sb[l]), (pu, wu_sb[l])):
                        for j in range(3):
                            rows = min(128, FS - j * 128)
                            for k in range(KT):
                                nc.tensor.matmul(
                                    out=ps[:rows, j * 512:j * 512 + PT],
                                    lhsT=wsb[:, k * FS + j * 128:
                                             k * FS + j * 128 + rows],
                                    rhs=xb2[:, k * PT:(k + 1) * PT],
                                    start=(k == 0), stop=(k == KT - 1))
                    gs = sb.tile([128, 3 * PT], F32, tag="gsf", name="gsf", bufs=1)
                    gsv = gs[:].rearrange("p (g c) -> p g c", g=3)
                    nc.scalar.activation(gsv, pgv, AF.Sigmoid)
                    nc.vector.tensor_tensor(out=gsv, in0=gsv, in1=pgv, op=ALU.mult)
                    af = sb.tile([128, 3 * PT], BF16, tag="af", name="af", bufs=1)
                    nc.vector.tensor_tensor(
                        out=af[:].rearrange("p (g c) -> p g c", g=3),
                        in0=gsv, in1=puv, op=ALU.mult)
                with psum_pool("pdnp") as pp:
                    pd = pp.tile([128, KT * 512], F32, tag="pbig", name="pdd")
                    for m in range(KT):
                        for j in range(3):
                            nc.tensor.matmul(
                                out=pd[:, m * 512:m * 512 + PT],
                                lhsT=wd_sb[l][:, j * D + m * 128:
                                              j * D + (m + 1) * 128],
                                rhs=af[:, j * PT:(j + 1) * PT],
                                start=(j == 0), stop=(j == 2))
                    ar_big(pd, hT)

            # last-token hidden -> column form [128, 2k+b]
            hT2 = hb.tile([128, 2 * KT], F32, tag="hT2", name="hT2_0")
            nc.vector.tensor_copy(
                hT2[:],
                hT[:].rearrange("p (q c) -> p q c", q=2 * KT)[:, :, L - 1:L])

            # ============ column-form helpers ============
            def rms_col(h):
                """h [128, 2k+b] f32 -> x bf16 same layout."""
                sq = sb.tile([128, 2 * KT], F32, tag="dsq", name="dsq")
                nc.vector.tensor_tensor(out=sq[:], in0=h[:], in1=h[:], op=ALU.mult)
                x = sb.tile([128, 2 * KT], BF16, tag="dx", name="dx")
                with psum_pool("pcrms") as pp:
                    pssd = pp.tile([1, 2 * KT], F32, tag="pssd", name="pssd")
                    nc.tensor.matmul(out=pssd[:], lhsT=ones[:], rhs=sq[:],
                                     start=True, stop=True)
                    sums = sb.tile([1, B], F32, tag="dsums", name="dsums")
                    nc.vector.tensor_reduce(
                        sums[:], pssd[0:1, :].rearrange("p (k b) -> p b k", k=KT),
                        AX.X, ALU.add)
                    sd = sb.tile([1, B], F32, tag="dsd", name="dsd")
                    nc.scalar.activation(sd[:], sums[:], AF.Sqrt, bias=epsc[:],
                                         scale=1.0 / D)
                    r = sb.tile([1, B], F32, tag="drn", name="drn")
                    nc.vector.reciprocal(r[:], sd[:])
                    rbc = pp.tile([128, B], F32, tag="drbc", name="drbc")
                    nc.tensor.matmul(out=rbc[:], lhsT=onesr[:], rhs=r[:],
                                     start=True, stop=True)
                    for k in range(KT):
                        nc.vector.tensor_tensor(out=x[:, 2 * k:2 * k + 2],
                                                in0=h[:, 2 * k:2 * k + 2],
                                                in1=rbc[:], op=ALU.mult)
                return x

            def ar_col(psum_col, hin, tag):
                """AllReduce column partial [128, 16] and return hin + g."""
                ev = sb.tile([128, 2 * KT], F32, tag="dev", name=f"dev{tag}")
                nc.vector.tensor_copy(
                    ev[:].rearrange("p (k c) -> p k c", k=KT),
                    psum_col[:].rearrange("p (k c) -> p k c", k=KT)[:, :, 0:2])
                bi = dp.tile([128, 2 * KT], F32, tag=f"{tag}i", name=f"{tag}i")
                bo = dp.tile([128, 2 * KT], F32, tag=f"{tag}o", name=f"{tag}o")
                nc.sync.dma_start(bi[:], ev[:])
                nc.gpsimd.collective_compute("AllReduce", ALU.add, replica_groups=RG,
                                             ins=[bi[:].opt()], outs=[bo[:].opt()])
                g = sb.tile([128, 2 * KT], F32, tag="dgt", name=f"dg{tag}")
                nc.sync.dma_start(g[:], bo[:])
                h2 = hb.tile([128, 2 * KT], F32, tag="hT2", name=f"h2{tag}")
                nc.vector.tensor_tensor(out=h2[:], in0=hin[:], in1=g[:], op=ALU.add)
                return h2

            # ============ decode layer ============
            def decode_layer(l, hin, t):
                slen = L + t
                x = rms_col(hin)
                qk2 = sb.tile([B, 2 * QC], F32, tag="dqk2", name="dqk2")
                vrow = sb.tile([B, QC], BF16, tag="dvrow", name="dvrow")
                with psum_pool("pdq") as pp:
                    # qkv row-form [2, 384]: q 0:128, k 128:256, v 256:384
                    pq = pp.tile([B, 3 * QC], F32, tag="dpq", name="dpq")
                    for k in range(KT):
                        nc.tensor.matmul(
                            out=pq[:],
                            lhsT=x[:, 2 * k:2 * k + 2],
                            rhs=wqkv_sb[l][:, k * 3 * QC:(k + 1) * 3 * QC],
                            start=(k == 0), stop=(k == KT - 1))
                    # rope on q|k [2, 256]
                    rot = sb.tile([B, 2 * QC], F32, tag="drot", name="drot")
                    pqv = pq[:, 0:2 * QC].rearrange("p (blk h c) -> p blk h c",
                                                    blk=4, h=2)
                    rotv = rot[:].rearrange("p (blk h c) -> p blk h c", blk=4, h=2)
                    nc.scalar.activation(rotv[:, :, 0:1, :], pqv[:, :, 1:2, :],
                                         AF.Copy)
                    nc.scalar.activation(rotv[:, :, 1:2, :], pqv[:, :, 0:1, :],
                                         AF.Copy)
                    t1 = sb.tile([B, 2 * QC], F32, tag="dt1", name="dt1")
                    nc.vector.tensor_tensor(
                        out=t1[:], in0=pq[:, 0:2 * QC],
                        in1=dcosW[:, (t - 1) * 4 * HD:t * 4 * HD], op=ALU.mult)
                    nc.vector.tensor_tensor(
                        out=rot[:], in0=rot[:],
                        in1=dsinW[:, (t - 1) * 4 * HD:t * 4 * HD], op=ALU.mult)
                    nc.vector.tensor_tensor(out=qk2[:], in0=t1[:], in1=rot[:],
                                            op=ALU.add)
                    nc.vector.tensor_copy(vrow[:], pq[:, 2 * QC:3 * QC])
                nc.sync.dma_start(v1_c[l][t - 1:t, :], vrow[:])
                # transpose -> qkT [128, 4]: cols q0 q1 k0 k1
                qkT = sb.tile([128, 4], BF16, tag="dqkT", name="dqkT")
                with psum_pool("pdt") as pp:
                    pqt = pp.tile([128, 1024], F32, tag="dpqt", name="dpqt")
                    nc.tensor.transpose(pqt[:, 0:2], qk2[:, 0:QC], idf[:B, :B])
                    nc.tensor.transpose(pqt[:, 512:514], qk2[:, QC:2 * QC],
                                        idf[:B, :B])
                    nc.scalar.activation(
                        qkT[:].rearrange("p (g c) -> p g c", g=2),
                        pqt[:].rearrange("p (g c) -> p g c", g=2)[:, :, 0:2],
                        AF.Copy)
                for b in range(B):
                    nc.scalar.activation(
                        kT_c[l][:, b * SMAX + slen - 1:b * SMAX + slen],
                        qkT[:, 2 + b:3 + b], AF.Copy)
                # attention
                doT = sb.tile([128, B], BF16, tag="ddoT", name="ddoT")
                with psum_pool("pdattn") as pp:
                    psc = pp.tile([1, 4 * 512], F32, tag="dpsc", name="dpsc")
                    for b in range(B):
                        for h in range(HC):
                            p = 2 * b + h
                            nc.tensor.matmul(
                                out=psc[0:1, p * 512:p * 512 + slen],
                                lhsT=qkT[h * HD:(h + 1) * HD, b:b + 1],
                                rhs=kT_c[l][h * HD:(h + 1) * HD,
                                            b * SMAX:b * SMAX + slen],
                                start=True, stop=True)
                    esb = sb.tile([1, 4 * SMAX], F32, tag="desb", name="desb")
                    nc.scalar.activation(
                        esb[:].rearrange("p (q c) -> p q c", q=4)[:, :, 0:slen],
                        psc[:].rearrange("p (q c) -> p q c", q=4)[:, :, 0:slen],
                        AF.Exp, scale=0.125)
                    ssum = sb.tile([1, 4], F32, tag="dss", name="dss")
                    nc.vector.tensor_reduce(
                        ssum[:],
                        esb[:].rearrange("p (q c) -> p q c", q=4)[:, :, 0:slen],
                        AX.X, ALU.add)
                    rr = sb.tile([1, 4], F32, tag="drr", name="drr")
                    nc.vector.reciprocal(rr[:], ssum[:])
                    att = sb.tile([1, 4 * SMAX], BF16, tag="datt", name="datt")
                    for p in range(4):
                        nc.vector.tensor_scalar_mul(
                            att[0:1, p * SMAX:p * SMAX + slen],
                            esb[0:1, p * SMAX:p * SMAX + slen], rr[0:1, p:p + 1])
                    pat = pp.tile([L, 4], F32, tag="dpat", name="dpat")
                    pat1 = pp.tile([8, 4], F32, tag="dpat1", name="dpat1")
                    attf = sb.tile([1, 4 * SMAX], F32, tag="dattf", name="dattf")
                    nc.vector.tensor_copy(attf[:], att[:])
                    for p in range(4):
                        nc.tensor.transpose(pat[:, p:p + 1],
                                            attf[0:1, p * SMAX:p * SMAX + L],
                                            idf[:1, :1])
                        nc.tensor.transpose(pat1[0:t, p:p + 1],
                                            attf[0:1, p * SMAX + L:p * SMAX + slen],
                                            idf[:1, :1])
                    attT = sb.tile([L, 4], BF16, tag="dattT", name="dattT")
                    nc.scalar.activation(attT[:], pat[:], AF.Copy)
                    attT1 = sb.tile([8, 4], BF16, tag="dattT1", name="dattT1")
                    nc.scalar.activation(attT1[0:t, :], pat1[0:t, :], AF.Copy)
                    pov = pp.tile([HD, 4], F32, tag="dpov", name="dpov")
                    for b in range(B):
                        for h in range(HC):
                            p = 2 * b + h
                            nc.tensor.matmul(out=pov[:, p:p + 1],
                                             lhsT=v0_c[l][:, p * HD:(p + 1) * HD],
                                             rhs=attT[:, p:p + 1], start=True,
                                             stop=False)
                            nc.tensor.matmul(out=pov[:, p:p + 1],
                                             lhsT=v1_c[l][0:t, p * HD:(p + 1) * HD],
                                             rhs=attT1[0:t, p:p + 1], start=False,
                                             stop=True)
                    for b in range(B):
                        for h in range(HC):
                            p = 2 * b + h
                            nc.scalar.activation(doT[h * HD:(h + 1) * HD, b:b + 1],
                                                 pov[:, p:p + 1], AF.Copy)
                # wo column-out [128, 2k+b]
                with psum_pool("pdwo") as pp:
                    pwo = pp.tile([128, KT * 512], F32, tag="dpwo", name="dpwo")
                    for j in range(KT):
                        nc.tensor.matmul(out=pwo[:, j * 512:j * 512 + 2],
                                         lhsT=wo_sb[l][:, j * 128:(j + 1) * 128],
                                         rhs=doT[:], start=True, stop=True)
                    h2 = ar_col(pwo, hin, f"A{l}")
                # FFN
                x2 = rms_col(h2)
                af = sb.tile([B, FS], BF16, tag="daf", name="daf")
                with psum_pool("pdffn") as pp:
                    pg = pp.tile([B, FS], F32, tag="dpg", name="dpg")
                    pu = pp.tile([B, FS], F32, tag="dpu", name="dpu")
                    for ps, wsb in ((pg, wg_sb[l]), (pu, wu_sb[l])):
                        for k in range(KT):
                            nc.tensor.matmul(out=ps[:],
                                             lhsT=x2[:, 2 * k:2 * k + 2],
                                             rhs=wsb[:, k * FS:(k + 1) * FS],
                                             start=(k == 0), stop=(k == KT - 1))
                    gs = sb.tile([B, FS], F32, tag="dgs", name="dgs")
                    nc.scalar.activation(gs[:], pg[:], AF.Sigmoid)
                    nc.vector.tensor_tensor(out=gs[:], in0=gs[:], in1=pg[:],
                                            op=ALU.mult)
                    nc.vector.tensor_tensor(out=af[:], in0=gs[:], in1=pu[:],
                                            op=ALU.mult)
                # transpose af -> aT [128, (j, b)]
                aT = sb.tile([128, 6], BF16, tag="daT", name="daT")
                aff = sb.tile([B, FS], F32, tag="daff", name="daff")
                nc.vector.tensor_copy(aff[:], af[:])
                with psum_pool("pdaT") as pp:
                    pat3 = pp.tile([128, 6], F32, tag="dpat3", name="dpat3")
                    for j in range(3):
                        rows = min(128, FS - j * 128)
                        nc.tensor.transpose(pat3[:rows, 2 * j:2 * j + 2],
                                            aff[:, j * 128:j * 128 + rows],
                                            idf[:B, :B])
                    nc.scalar.activation(aT[:], pat3[:], AF.Copy)
                # down column-out (wd host-padded to 128 rows per chunk)
                with psum_pool("pdd") as pp:
                    pd = pp.tile([128, KT * 512], F32, tag="dpd", name="dpd")
                    for m in range(KT):
                        for j in range(3):
                            nc.tensor.matmul(
                                out=pd[:, m * 512:m * 512 + 2],
                                lhsT=wd_sb[l][:, j * D + m * 128:
                                              j * D + (m + 1) * 128],
                                rhs=aT[:, 2 * j:2 * j + 2],
                                start=(j == 0), stop=(j == 2))
                    h3 = ar_col(pd, h2, f"F{l}")
                return h3

            # ============ vocab step ============
            def vocab_step(tt, hcol):
                xf = rms_col(hcol)
                ll = sb.tile([B, VS], F32, tag="big8k", name="ll", bufs=1)
                lsum8 = sb.tile([B, 8], F32, tag="lsum8", name="lsum8")
                with psum_pool("plmp") as pp:
                    plms = [pp.tile([B, VCW], F32, tag=f"plm{v}", name=f"plm{v}")
                            for v in range(4)]
                    for v in range(8):
                        pv = plms[v % 4]
                        for k in range(KT):
                            nc.tensor.matmul(
                                out=pv[:],
                                lhsT=xf[:, 2 * k:2 * k + 2],
                                rhs=lmh_sb[:, k * VS + v * VCW:
                                           k * VS + (v + 1) * VCW],
                                start=(k == 0), stop=(k == KT - 1))
                        nc.scalar.activation(ll[:, v * VCW:(v + 1) * VCW], pv[:],
                                             AF.Copy, accum_out=lsum8[:, v:v + 1])
                lsum = sb.tile([B, 1], F32, tag="lsum", name="lsum")
                nc.vector.tensor_reduce(lsum[:], lsum8[:], AX.X, ALU.add)
                m8 = sb.tile([B, 8], F32, tag="m8", name="m8")
                nc.vector.max(m8[:], ll[:])
                i8 = sb.tile([B, 8], U32, tag="i8", name="i8")
                nc.vector.max_index(i8[:], m8[:], ll[:])
                idxf = sb.tile([B, 1], F32, tag="idxf", name="idxf")
                nc.vector.tensor_copy(idxf[:], i8[:, 0:1])
                gidx = sb.tile([B, 1], F32, tag="gidx", name="gidx")
                nc.vector.tensor_tensor(out=gidx[:], in0=idxf[:], in1=coreoff[:],
                                        op=ALU.add)
                # stats [2, 3]: max, gidx, lsum -> AG -> [8, 6]
                st = sb.tile([B, 4], F32, tag="st", name="st")
                nc.vector.tensor_copy(st[:, 0:1], m8[:, 0:1])
                nc.vector.tensor_copy(st[:, 1:2], gidx[:])
                nc.vector.tensor_copy(st[:, 2:3], lsum[:])
                nc.vector.memset(st[:, 3:4], 0.0)
                sbi = dp.tile([1, 8], F32, tag="sti", name="sti")
                sbo = dp.tile([8, 8], F32, tag="sto", name="sto")
                nc.sync.dma_start(sbi[:], st[:])     # flatten: col = 3b + s
                nc.gpsimd.collective_compute("AllGather", ALU.bypass,
                                             replica_groups=RG,
                                             ins=[sbi[:].opt()], outs=[sbo[:].opt()])
                gsr = sb.tile([1, 64], F32, tag="gsr", name="gsr")
                nc.sync.dma_start(gsr[:], sbo[:])
                # col layout: 8*core + 4*b + {0 max, 1 idx, 2 sum, 3 pad}
                gv = gsr[0:1, :].rearrange("p (c s) -> p s c", s=8)
                nidrow = sb.tile([1, B], F32, tag="nidrow", name="nidrow")
                msumrow = sb.tile([1, B], F32, tag="msumrow", name="msumrow")
                for b in range(B):
                    rmax = sb.tile([1, 1], F32, tag="rmax", name=f"rmax{b}")
                    nc.vector.tensor_reduce(rmax[:], gv[:, 4 * b:4 * b + 1, :],
                                            AX.X, ALU.max)
                    mka = sb.tile([1, 8], U32, tag="mka", name=f"mka{b}")
                    nc.vector.tensor_scalar(
                        out=mka[0:1, :].rearrange("p (x c) -> p x c", x=1),
                        in0=gv[:, 4 * b:4 * b + 1, :],
                        scalar1=rmax[:], scalar2=None, op0=ALU.is_equal)
                    cand = sb.tile([1, 8], F32, tag="cand", name=f"cand{b}")
                    nc.vector.select(
                        cand[0:1, :].rearrange("p (x c) -> p x c", x=1),
                        mka[0:1, :].rearrange("p (x c) -> p x c", x=1),
                        gv[:, 4 * b + 1:4 * b + 2, :],
                        big18[0:1, :].rearrange("p (x c) -> p x c", x=1))
                    nc.vector.tensor_reduce(nidrow[0:1, b:b + 1], cand[:], AX.X,
                                            ALU.min)
                    nc.vector.tensor_reduce(msumrow[0:1, b:b + 1],
                                            gv[:, 4 * b + 2:4 * b + 3, :],
                                            AX.X, ALU.add)
                nid = sb.tile([B, 1], F32, tag="nid", name="nid")
                msc = sb.tile([B, 1], F32, tag="msc", name="msc")
                with psum_pool("pnmp") as pp:
                    pnid = pp.tile([B, 1], F32, tag="pnid", name="pnid")
                    nc.tensor.transpose(pnid[:], nidrow[:], idf[:1, :1])
                    nc.vector.tensor_copy(nid[:], pnid[:])
                    pms = pp.tile([B, 1], F32, tag="pms", name="pms")
                    nc.tensor.transpose(pms[:], msumrow[:], idf[:1, :1])
                    nc.scalar.activation(msc[:], pms[:], AF.Copy, scale=1.0 / V)
                clrs = sb.tile([B, HOUT], F32, tag="xnorm", name="clrs", bufs=1)
                nc.vector.tensor_scalar(out=clrs[:], in0=ll[:, 0:HOUT],
                                        scalar1=msc[:], scalar2=None,
                                        op0=ALU.subtract)
                nc.sync.dma_start(out_t[0:B, tt:tt + 1, :], clrs[:])
                if tt == T_NEW - 1:
                    return None
                if tt == 0:
                    nc.sync.dma_start(dbg_st[:], st[:])
                    nc.sync.dma_start(dbg_gsr[:], gsr[:])
                    nc.sync.dma_start(dbg_nid[:], nid[:])
                    nc.sync.dma_start(dbg_m8[:], m8[:])
                    nc.sync.dma_start(dbg_ll[:], ll[:, 0:1000])
                nidu = sb.tile([B, 1], U32, tag="nidu", name="nidu")
                nc.vector.tensor_copy(nidu[:], nid[:])
                embrow = sb.tile([B, D], F32, tag="gsf", name="embrow", bufs=1)
                nc.gpsimd.indirect_dma_start(
                    out=embrow[:], out_offset=None, in_=emb_in[:, :],
                    in_offset=bass.IndirectOffsetOnAxis(ap=nidu[:, 0:1], axis=0),
                    bounds_check=V - 1, oob_is_err=False)
                hnew = hb.tile([128, 2 * KT], F32, tag="hT2", name=f"hemb{tt}")
                with psum_pool("pembp") as pp:
                    pemb = pp.tile([128, KT * 512], F32, tag="pemb", name="pemb")
                    for k in range(KT):
                        nc.tensor.transpose(pemb[:, k * 512:k * 512 + 2],
                                            embrow[:, k * 128:(k + 1) * 128],
                                            idf[:B, :B])
                    nc.vector.tensor_copy(
                        hnew[:].rearrange("p (k c) -> p k c", k=KT),
                        pemb[:].rearrange("p (k c) -> p k c", k=KT)[:, :, 0:2])
                return hnew

            nc.sync.dma_start(dbg_hT2[:], hT2[:])
            import os
            phase = int(os.environ.get("KPHASE", "0"))
            if phase != 1:
                hcur = vocab_step(0, hT2)
                tmax = T_NEW if phase == 0 else phase
                for t in range(1, tmax):
                    for l in range(NL):
                        hcur = decode_layer(l, hcur, t)
                    if phase == 0 or t < tmax - 1:
                        hcur = vocab_step(t, hcur)

    nc.compile()
    return nc


def make_in_maps(inputs):
    import ml_dtypes
    bf = ml_dtypes.bfloat16
    ii = {k: np.asarray(v) for k, v in inputs.items()}
    embed = ii["embed"].astype(np.float32)
    tokens = ii["input_ids"].astype(np.int64)
    h0 = embed[tokens]                                   # [B, L, D]
    # column layout [128, (k, b, t)]
    h0T = np.ascontiguousarray(
        h0.transpose(2, 0, 1).reshape(KT, 128, B * L).transpose(1, 0, 2)
        .reshape(128, KT * PT)).astype(np.float32)

    inv = ROPE_BASE ** (-np.arange(32, dtype=np.float64) / 32)
    dd = np.arange(HD)
    sgn = np.where(dd < 32, -1.0, 1.0)
    fr = inv[dd % 32]

    pos_p = np.tile(np.arange(L), B)
    ang_p = np.outer(fr, pos_p)
    pcosT = np.tile(np.cos(ang_p), (2, 1)).astype(np.float32)
    psinT = np.tile(sgn[:, None] * np.sin(ang_p), (2, 1)).astype(np.float32)

    pos_d = np.arange(L, L + T_NEW - 1)
    ang_d = np.outer(pos_d, fr)
    dcosW = np.tile(np.tile(np.cos(ang_d), (1, 4)).reshape(1, -1),
                    (B, 1)).astype(np.float32)
    dsinW = np.tile(np.tile(np.sin(ang_d) * sgn[None, :], (1, 4)).reshape(1, -1),
                    (B, 1)).astype(np.float32)

    q_idx = np.arange(L)[:, None]
    cmask = np.where(np.arange(L)[None, :] <= q_idx, 0.0, -8e9).astype(np.float32)
    cmask4 = np.tile(cmask, (1, 4)).astype(np.float32)
    idf = np.eye(128, dtype=np.float32)
    idb = np.eye(128).astype(bf)

    def col_chunks(w):
        # [D, C] -> [128, (k, C)]
        Dn, C = w.shape
        return np.ascontiguousarray(
            w.reshape(KT, 128, C).transpose(1, 0, 2).reshape(128, KT * C))

    an, fn, fin = ii["attn_norm"], ii["ffn_norm"], ii["final_norm"]
    in_maps = []
    for c in range(TP):
        m = {"h0T": h0T, "emb": embed, "pcosT": pcosT, "psinT": psinT,
             "dcosW": dcosW, "dsinW": dsinW, "cmask4": cmask4, "idf": idf,
             "idb": idb, "coreoff": np.full((B, 1), c * VS, np.float32)}
        for l in range(NL):
            ws = []
            for key in ("wq", "wk", "wv"):
                ws.append((an[l][:, None] * ii[key][l])[:, c * QC:(c + 1) * QC])
            # interleave per k: [q|k|v]
            qkv = np.stack(ws, axis=0)                    # [3, D, QC]
            qkv = qkv.reshape(3, KT, 128, QC).transpose(2, 1, 0, 3)
            m[f"wqkv{l}"] = np.ascontiguousarray(
                qkv.reshape(128, KT * 3 * QC)).astype(bf)
            m[f"wo{l}"] = np.ascontiguousarray(
                ii["wo"][l][c * QC:(c + 1) * QC, :]).astype(bf)
            m[f"wg{l}"] = col_chunks(
                (fn[l][:, None] * ii["w_gate"][l])[:, c * FS:(c + 1) * FS]).astype(bf)
            m[f"wu{l}"] = col_chunks(
                (fn[l][:, None] * ii["w_up"][l])[:, c * FS:(c + 1) * FS]).astype(bf)
            wd = ii["w_down"][l][c * FS:(c + 1) * FS, :]
            wdp = np.zeros((3 * 128, D), np.float32)
            wdp[:FS] = wd
            m[f"wd{l}"] = np.ascontiguousarray(
                wdp.reshape(3, 128, D).transpose(1, 0, 2).reshape(128, 3 * D)
            ).astype(bf)
        m["lmh"] = col_chunks(
            (fin[:, None] * ii["lm_head"])[:, c * VS:(c + 1) * VS]).astype(bf)
        in_maps.append(m)
    return in_maps


_NC_CACHE = {}


def kernel(**inputs):
    if "nc" not in _NC_CACHE:
        _NC_CACHE["nc"] = build()
    nc = _NC_CACHE["nc"]
    in_maps = make_in_maps(inputs)
    res = bass_utils.run_bass_kernel_spmd(nc, in_maps, core_ids=list(range(TP)))
    return np.asarray(res.results[0]["out"], dtype=np.float32)
